# revision 1
# baseline (speedup 1.0000x reference)
"""Global-attention kernel for [8, 384, 32, 32] ConvAttention on 8 trn2 cores.

Math (per reference): tokens over B*H*W = 8192 positions, C = 384 channels
split as V/K/Q of 128 each; out = softmax(Q K^T / sqrt(128)) V, re-laid as
[B, 128, H, W].

Sharding: core c owns the 1024 query tokens of batch c (token n = b*1024+hw,
so batch == contiguous token block). K/V are replicated. Each core computes
its row block of the attention entirely locally; no collectives.

On-core layout: everything channel-major ([d, token]) which is exactly how
x is laid out in DRAM, so host prep is just slicing + two cheap transposes:
  qT [128, 1024]  = x[c, 256:384].reshape(128, 1024)          (per core)
  kT [128, 8192]  = x[:, 128:256] tokens, channel-major        (replicated)
  vt [128, 8192]  = V tokens chunk-transposed: vt[p, 128*j+v] = V[128*j+p, v]
The S^T = K_chunk Q^T matmul then needs no on-chip transposes at all, the
softmax denominator comes from a ones-vector matmul (partition reduction on
PE), and the output lands directly in [v, token] = DRAM layout.
"""

import math

import numpy as np

import concourse.bass as bass
import concourse.tile as tile
from concourse import bacc, mybir
from concourse.bass_utils import run_bass_kernel_spmd

N_CORES = 8
B, C, H, W = 8, 384, 32, 32
HW = H * W            # 1024 tokens per batch == per core
N = B * HW            # 8192 total tokens
D = 128               # key/value width
NCHUNK = N // 128     # 64 kv chunks of 128 tokens
SCALE = 1.0 / math.sqrt(D)
F32 = mybir.dt.float32
F32R = mybir.dt.float32r

# Rowsum work split: chunks 0..RS_PE_CHUNKS-1 reduce on PE (ones-matmul),
# the rest accumulate elementwise on the otherwise-idle DVE and get folded
# in with one final ones-matmul.
RS_PE_CHUNKS = 64  # v1: all on PE; tune later


def _build_nc():
    nc = bacc.Bacc(
        "TRN2", target_bir_lowering=False, debug=False, num_devices=N_CORES
    )
    qT = nc.dram_tensor("qT", [D, HW], F32, kind="ExternalInput").ap()
    kT = nc.dram_tensor("kT", [D, N], F32, kind="ExternalInput").ap()
    vt = nc.dram_tensor("vt", [D, N], F32, kind="ExternalInput").ap()
    ones = nc.dram_tensor("ones", [D, 1], F32, kind="ExternalInput").ap()
    oT = nc.dram_tensor("oT", [D, HW], F32, kind="ExternalOutput").ap()

    with tile.TileContext(nc) as tc:
        with (
            tc.tile_pool(name="persist", bufs=1) as persist,
            tc.tile_pool(name="etile", bufs=6) as epool,
            tc.tile_pool(name="spsum", bufs=2, space="PSUM") as spsum,
            tc.tile_pool(name="apsum", bufs=1, space="PSUM") as apsum,
        ):
            qT_sb = persist.tile([D, HW], F32R, tag="qT_sb")
            ones_sb = persist.tile([D, 1], F32R, tag="ones_sb")
            kT_sb = [persist.tile([D, HW], F32R, tag=f"kT{i}", name=f"kT_sb{i}") for i in range(8)]
            vt_sb = [persist.tile([D, HW], F32R, tag=f"vt{i}", name=f"vt_sb{i}") for i in range(8)]

            nc.sync.dma_start(out=qT_sb[:], in_=qT[:].bitcast(F32R))
            nc.sync.dma_start(out=ones_sb[:], in_=ones[:].bitcast(F32R))
            # Interleave K/V pieces so PV(c) never waits behind the whole
            # K stream.
            for i in range(8):
                nc.sync.dma_start(out=kT_sb[i][:], in_=kT[:, i * HW : (i + 1) * HW].bitcast(F32R))
                nc.sync.dma_start(out=vt_sb[i][:], in_=vt[:, i * HW : (i + 1) * HW].bitcast(F32R))

            o_psum = apsum.tile([D, HW], F32, tag="o_psum")
            rs_psum = apsum.tile([1, HW], F32, tag="rs_psum")

            rs_acc = persist.tile([D, HW], F32, tag="rs_acc")
            nc.vector.memset(rs_acc[:], 0.0)

            def emit_qk(c):
                blk, off = c // 8, (c % 8) * 128
                s_ps = spsum.tile([D, HW], F32, tag="s_ps", name=f"s_ps{c}")
                for h in range(2):
                    nc.tensor.matmul(
                        s_ps[:, h * 512 : (h + 1) * 512],
                        kT_sb[blk][:, off : off + 128],
                        qT_sb[:, h * 512 : (h + 1) * 512],
                        start=True,
                        stop=True,
                    )
                return s_ps

            # Software-pipelined by one chunk: PE's program order is
            # QK(c+1) -> PV(c), so PE streams QK(c+1) while ACT exps S(c)
            # instead of stalling in-order behind PV(c)'s wait.
            s_tiles = {0: emit_qk(0)}
            first_pe_rs = True
            for c in range(NCHUNK):
                if c + 1 < NCHUNK:
                    s_tiles[c + 1] = emit_qk(c + 1)

                e_sb = epool.tile([D, HW], F32R, tag="e_sb", name=f"e_sb{c}")
                nc.scalar.activation(
                    e_sb[:],
                    s_tiles.pop(c)[:],
                    mybir.ActivationFunctionType.Exp,
                    scale=SCALE,
                )

                blk, off = c // 8, (c % 8) * 128
                for h in range(2):
                    nc.tensor.matmul(
                        o_psum[:, h * 512 : (h + 1) * 512],
                        vt_sb[blk][:, off : off + 128],
                        e_sb[:, h * 512 : (h + 1) * 512],
                        start=(c == 0),
                        stop=(c == NCHUNK - 1),
                    )

                # Rowsum: ~1/5 of chunks reduce on PE (ones-matmul), the rest
                # accumulate elementwise on the otherwise-idle DVE; balanced
                # so ACT's exp stream stays the critical path.
                if c % 5 == 4:
                    for h in range(2):
                        nc.tensor.matmul(
                            rs_psum[:, h * 512 : (h + 1) * 512],
                            ones_sb[:],
                            e_sb[:, h * 512 : (h + 1) * 512],
                            start=first_pe_rs,
                            stop=False,
                        )
                    first_pe_rs = False
                else:
                    nc.vector.tensor_add(
                        rs_acc[:], rs_acc[:], e_sb[:].bitcast(F32)
                    )

            # Fold the DVE partial sums into the PSUM rowsum (via an f32r
            # copy so the fp32r matmul verifier sees a rounded producer).
            rs_acc_r = persist.tile([D, HW], F32R, tag="rs_acc_r")
            nc.scalar.copy(rs_acc_r[:], rs_acc[:])
            for h in range(2):
                nc.tensor.matmul(
                    rs_psum[:, h * 512 : (h + 1) * 512],
                    ones_sb[:],
                    rs_acc_r[:, h * 512 : (h + 1) * 512],
                    start=first_pe_rs,
                    stop=True,
                )

            # softmax denominator -> reciprocal -> scale columns of o_psum
            rs_sb = persist.tile([1, HW], F32, tag="rs_sb")
            nc.scalar.copy(rs_sb[:], rs_psum[:])
            recip_sb = persist.tile([1, HW], F32, tag="recip_sb")
            nc.vector.reciprocal(recip_sb[:], rs_sb[:])
            bc_sb = persist.tile([D, HW], F32, tag="bc_sb")
            nc.gpsimd.partition_broadcast(bc_sb[:], recip_sb[:])
            o_sb = persist.tile([D, HW], F32, tag="o_sb")
            nc.vector.tensor_mul(o_sb[:], o_psum[:], bc_sb[:])
            nc.sync.dma_start(out=oT[:], in_=o_sb[:])

    nc.compile()
    return nc


_NC_CACHE = None


def _get_nc():
    global _NC_CACHE
    if _NC_CACHE is None:
        _NC_CACHE = _build_nc()
    return _NC_CACHE


def kernel(x: np.ndarray) -> np.ndarray:
    assert x.shape == (B, C, H, W), x.shape
    x = np.ascontiguousarray(x, dtype=np.float32)
    xr = x.reshape(B, C, HW)

    # K channel-major over all tokens: kT[d, b*1024+hw] = x[b, 128+d, hw]
    kT = np.ascontiguousarray(xr[:, 128:256, :].transpose(1, 0, 2)).reshape(D, N)
    # V chunk-transposed: vt[p, 128*j + v] = V[128*j + p, v],
    # V[n, v] = x[b, v, hw] with n = b*1024 + hw
    v_tok = np.ascontiguousarray(xr[:, 0:128, :].transpose(0, 2, 1)).reshape(N, D)
    vt = np.ascontiguousarray(v_tok.reshape(NCHUNK, 128, D).transpose(1, 0, 2)).reshape(
        D, N
    )

    ones_col = np.ones((D, 1), dtype=np.float32)
    in_maps = []
    for c in range(N_CORES):
        qT = np.ascontiguousarray(xr[c, 256:384, :])
        in_maps.append({"qT": qT, "kT": kT, "vt": vt, "ones": ones_col})

    nc = _get_nc()
    res = run_bass_kernel_spmd(nc, in_maps, list(range(N_CORES)))

    out = np.empty((B, D, H, W), dtype=np.float32)
    for c in range(N_CORES):
        out[c] = res.results[c]["oT"].reshape(D, H, W)
    return out



# revision 6
# speedup vs baseline: 1.0943x; 1.0943x over previous
"""Global-attention kernel for [8, 384, 32, 32] ConvAttention on 8 trn2 cores.

Math (per reference): tokens over B*H*W = 8192 positions, C = 384 channels
split as V/K/Q of 128 each; out = softmax(Q K^T / sqrt(128)) V, re-laid as
[B, 128, H, W].

Sharding: core c owns the 1024 query tokens of batch c (token n = b*1024+hw,
so batch == contiguous token block). K/V are replicated. Each core computes
its row block of the attention entirely locally; no collectives.

v2 engine balance (per 128-kv-token chunk, 64 chunks):
  PE : QK (2x512 mm) + PV (2x512 mm)          ~853 ns   (bf16, 1 cyc/row)
  ACT: exp [128,1024] PSUM->SBUF bf16         ~1038 ns  <- critical path
  DVE: rowsum via bf16 pair-add (533) + fp32 accumulate (1067) per pair
       ~800 ns/chunk
  PE rowsum/ones-matmuls removed from the main loop entirely; epilogue is
  pipelined in 4 column strips (fold-mm -> ACT copy -> recip -> PE
  broadcast-mm -> DVE mul -> DMA out).

All matmul inputs are bf16 (host-converted), halving input DMA; first K/V
pieces are small so the first QK starts ~3us in.
"""

import math

import numpy as np
import ml_dtypes

import concourse.bass as bass
import concourse.tile as tile
from concourse import bacc, mybir
from concourse.bass_utils import run_bass_kernel_spmd

N_CORES = 8
B, C, H, W = 8, 384, 32, 32
HW = H * W            # 1024 tokens per batch == per core
N = B * HW            # 8192 total tokens
D = 128               # key/value width
NCHUNK = N // 128     # 64 kv chunks of 128 tokens
SCALE = 1.0 / math.sqrt(D)
F32 = mybir.dt.float32
F32R = mybir.dt.float32r
BF16 = mybir.dt.bfloat16

NSTRIP = 4            # epilogue column strips
SW = HW // NSTRIP


def _build_nc():
    nc = bacc.Bacc(
        "TRN2", target_bir_lowering=False, debug=False, num_devices=N_CORES
    )
    qT = nc.dram_tensor("qT", [D, HW], BF16, kind="ExternalInput").ap()
    kT = nc.dram_tensor("kT", [D, N], BF16, kind="ExternalInput").ap()
    vt = nc.dram_tensor("vt", [D, N], BF16, kind="ExternalInput").ap()
    onesd = nc.dram_tensor("onesd", [D, 1], F32, kind="ExternalInput").ap()
    oT = nc.dram_tensor("oT", [D, HW], F32, kind="ExternalOutput").ap()

    with tile.TileContext(nc) as tc:
        with (
            tc.tile_pool(name="persist", bufs=1) as persist,
            tc.tile_pool(name="etile", bufs=6) as epool,
            tc.tile_pool(name="pair", bufs=2) as ppool,
            tc.tile_pool(name="spsum", bufs=2, space="PSUM") as spsum,
            tc.tile_pool(name="apsum", bufs=1, space="PSUM") as apsum,
        ):
            qT_sb = persist.tile([D, HW], BF16, tag="qT_sb")
            onesd_sb = persist.tile([D, 1], F32R, tag="onesd_sb")
            kT_sb = [persist.tile([D, HW], BF16, tag=f"kT{i}", name=f"kT_sb{i}") for i in range(8)]
            vt_sb = [persist.tile([D, HW], BF16, tag=f"vt{i}", name=f"vt_sb{i}") for i in range(8)]

            # Startup-latency-ordered DMA: Q first, then one small K piece
            # (2 chunks) and a small V piece so compute starts ~3us in, then
            # the bulk in [128,1024] pieces.
            nc.sync.dma_start(out=qT_sb[:], in_=qT[:])
            nc.sync.dma_start(out=onesd_sb[:], in_=onesd[:].bitcast(F32R))
            nc.sync.dma_start(out=kT_sb[0][:, 0:256], in_=kT[:, 0:256])
            nc.sync.dma_start(out=vt_sb[0][:, 0:256], in_=vt[:, 0:256])
            nc.sync.dma_start(out=kT_sb[0][:, 256:HW], in_=kT[:, 256:HW])
            nc.sync.dma_start(out=vt_sb[0][:, 256:HW], in_=vt[:, 256:HW])
            for i in range(1, 8):
                nc.sync.dma_start(out=kT_sb[i][:], in_=kT[:, i * HW : (i + 1) * HW])
                nc.sync.dma_start(out=vt_sb[i][:], in_=vt[:, i * HW : (i + 1) * HW])

            o_psum = apsum.tile([D, HW], F32, tag="o_psum")
            rs_ps = apsum.tile([1, HW], F32, tag="rs_ps")

            rs_acc = persist.tile([D, HW], F32, tag="rs_acc")
            nc.vector.memset(rs_acc[:], 0.0)

            def emit_qk(c):
                blk, off = c // 8, (c % 8) * 128
                s_ps = spsum.tile([D, HW], F32, tag="s_ps", name=f"s_ps{c}")
                for h in range(2):
                    nc.tensor.matmul(
                        s_ps[:, h * 512 : (h + 1) * 512],
                        kT_sb[blk][:, off : off + 128],
                        qT_sb[:, h * 512 : (h + 1) * 512],
                        start=True,
                        stop=True,
                    )
                return s_ps

            # Software-pipelined by one chunk: PE's program order is
            # QK(c+1) -> PV(c), so PE streams QK(c+1) while ACT exps S(c).
            s_tiles = {0: emit_qk(0)}
            e_tiles = {}
            for c in range(NCHUNK):
                if c + 1 < NCHUNK:
                    s_tiles[c + 1] = emit_qk(c + 1)

                e_sb = epool.tile([D, HW], BF16, tag="e_sb", name=f"e_sb{c}")
                nc.scalar.activation(
                    e_sb[:],
                    s_tiles.pop(c)[:],
                    mybir.ActivationFunctionType.Exp,
                    scale=SCALE,
                )
                e_tiles[c] = e_sb

                blk, off = c // 8, (c % 8) * 128
                for h in range(2):
                    nc.tensor.matmul(
                        o_psum[:, h * 512 : (h + 1) * 512],
                        vt_sb[blk][:, off : off + 128],
                        e_sb[:, h * 512 : (h + 1) * 512],
                        start=(c == 0),
                        stop=(c == NCHUNK - 1),
                    )

                # Rowsum on DVE only: bf16 pair-add (2x_1p mode) then fp32
                # accumulate, ~800 ns/chunk, under ACT's 1038 ns/chunk.
                if c % 2 == 1:
                    pair = ppool.tile([D, HW], BF16, tag="pair", name=f"pair{c}")
                    with nc.allow_low_precision(
                        reason="bf16 pair-sum of exp weights; accumulated in fp32"
                    ):
                        nc.vector.tensor_add(
                            pair[:], e_tiles.pop(c - 1)[:], e_tiles.pop(c)[:]
                        )
                        nc.vector.tensor_add(rs_acc[:], rs_acc[:], pair[:])

            # ---- epilogue: softmax denominator + normalize, 4 strips ----
            rs_acc_r = persist.tile([D, HW], F32R, tag="rs_acc_r")
            rs_sb = persist.tile([1, HW], F32, tag="rs_sb")
            recip = persist.tile([1, HW], F32, tag="recip")
            bc_sb = persist.tile([D, HW], F32, tag="bc_sb")
            o_sb = persist.tile([D, HW], F32, tag="o_sb")
            for s4 in range(NSTRIP):
                sl = slice(SW * s4, SW * (s4 + 1))
                # f32r copy so the fp32r matmul verifier sees a rounded
                # producer, then fold partition-sums of rs_acc on PE
                nc.scalar.copy(rs_acc_r[:, sl], rs_acc[:, sl])
                nc.tensor.matmul(
                    rs_ps[:, sl],
                    onesd_sb[:],
                    rs_acc_r[:, sl],
                    start=True,
                    stop=True,
                )
                nc.scalar.copy(rs_sb[:, sl], rs_ps[:, sl])
                nc.vector.reciprocal(recip[:, sl], rs_sb[:, sl])
                nc.gpsimd.partition_broadcast(bc_sb[:, sl], recip[:, sl])
                nc.vector.tensor_mul(o_sb[:, sl], o_psum[:, sl], bc_sb[:, sl])
                nc.sync.dma_start(out=oT[:, sl], in_=o_sb[:, sl])

    nc.compile()
    return nc


_NC_CACHE = None


def _get_nc():
    global _NC_CACHE
    if _NC_CACHE is None:
        _NC_CACHE = _build_nc()
    return _NC_CACHE


def _prep_inputs(x: np.ndarray):
    x = np.ascontiguousarray(x, dtype=np.float32)
    xr = x.reshape(B, C, HW)

    # K channel-major over all tokens: kT[d, b*1024+hw] = x[b, 128+d, hw]
    kT = np.ascontiguousarray(xr[:, 128:256, :].transpose(1, 0, 2)).reshape(D, N)
    # V chunk-transposed: vt[p, 128*j + v] = V[128*j + p, v],
    # V[n, v] = x[b, v, hw] with n = b*1024 + hw
    v_tok = np.ascontiguousarray(xr[:, 0:128, :].transpose(0, 2, 1)).reshape(N, D)
    vt = np.ascontiguousarray(v_tok.reshape(NCHUNK, 128, D).transpose(1, 0, 2)).reshape(
        D, N
    )
    kT = kT.astype(ml_dtypes.bfloat16)
    vt = vt.astype(ml_dtypes.bfloat16)

    onesd = np.ones((D, 1), dtype=np.float32)
    in_maps = []
    for c in range(N_CORES):
        qT = np.ascontiguousarray(xr[c, 256:384, :]).astype(ml_dtypes.bfloat16)
        in_maps.append(
            {"qT": qT, "kT": kT, "vt": vt, "onesd": onesd}
        )
    return in_maps


def kernel(x: np.ndarray) -> np.ndarray:
    assert x.shape == (B, C, H, W), x.shape
    in_maps = _prep_inputs(x)
    nc = _get_nc()
    res = run_bass_kernel_spmd(nc, in_maps, list(range(N_CORES)))

    out = np.empty((B, D, H, W), dtype=np.float32)
    for c in range(N_CORES):
        out[c] = np.asarray(res.results[c]["oT"], dtype=np.float32).reshape(D, H, W)
    return out


# revision 7
# speedup vs baseline: 1.1608x; 1.0608x over previous
"""Global-attention kernel for [8, 384, 32, 32] ConvAttention on 8 trn2 cores.

Math (per reference): tokens over B*H*W = 8192 positions, C = 384 channels
split as V/K/Q of 128 each; out = softmax(Q K^T / sqrt(128)) V, re-laid as
[B, 128, H, W].

Sharding: core c owns the 1024 query tokens of batch c (token n = b*1024+hw,
so batch == contiguous token block). K/V are replicated. Each core computes
its row block of the attention entirely locally; no collectives.

v2 engine balance (per 128-kv-token chunk, 64 chunks):
  PE : QK (2x512 mm) + PV (2x512 mm)          ~853 ns   (bf16, 1 cyc/row)
  ACT: exp [128,1024] PSUM->SBUF bf16         ~1038 ns  <- critical path
  DVE: rowsum via bf16 pair-add (533) + fp32 accumulate (1067) per pair
       ~800 ns/chunk
  PE rowsum/ones-matmuls removed from the main loop entirely; epilogue is
  pipelined in 4 column strips (fold-mm -> ACT copy -> recip -> PE
  broadcast-mm -> DVE mul -> DMA out).

All matmul inputs are bf16 (host-converted), halving input DMA; first K/V
pieces are small so the first QK starts ~3us in.
"""

import math

import numpy as np
import ml_dtypes

import concourse.bass as bass
import concourse.tile as tile
from concourse import bacc, mybir
from concourse.bass_utils import run_bass_kernel_spmd

N_CORES = 8
B, C, H, W = 8, 384, 32, 32
HW = H * W            # 1024 tokens per batch == per core
N = B * HW            # 8192 total tokens
D = 128               # key/value width
NCHUNK = N // 128     # 64 kv chunks of 128 tokens
SCALE = 1.0 / math.sqrt(D)
F32 = mybir.dt.float32
F32R = mybir.dt.float32r
BF16 = mybir.dt.bfloat16

NSTRIP = 4            # epilogue column strips
SW = HW // NSTRIP


def _build_nc():
    nc = bacc.Bacc(
        "TRN2", target_bir_lowering=False, debug=False, num_devices=N_CORES
    )
    qT = nc.dram_tensor("qT", [D, HW], BF16, kind="ExternalInput").ap()
    kT = nc.dram_tensor("kT", [D, N], BF16, kind="ExternalInput").ap()
    vt = nc.dram_tensor("vt", [D, N], BF16, kind="ExternalInput").ap()
    onesd = nc.dram_tensor("onesd", [D, 1], F32, kind="ExternalInput").ap()
    oT = nc.dram_tensor("oT", [D, HW], F32, kind="ExternalOutput").ap()

    with tile.TileContext(nc) as tc:
        with (
            tc.tile_pool(name="persist", bufs=1) as persist,
            tc.tile_pool(name="etile", bufs=6) as epool,
            tc.tile_pool(name="pair", bufs=2) as ppool,
            tc.tile_pool(name="spsum", bufs=3, space="PSUM") as spsum,
            tc.tile_pool(name="apsum", bufs=1, space="PSUM") as apsum,
        ):
            qT_sb = persist.tile([D, HW], BF16, tag="qT_sb")
            onesd_sb = persist.tile([D, 1], F32R, tag="onesd_sb")
            kT_sb = [persist.tile([D, HW], BF16, tag=f"kT{i}", name=f"kT_sb{i}") for i in range(8)]
            vt_sb = [persist.tile([D, HW], BF16, tag=f"vt{i}", name=f"vt_sb{i}") for i in range(8)]

            # Startup-latency-ordered DMA: Q first, then one small K piece
            # (2 chunks) and a small V piece so compute starts ~3us in, then
            # the bulk in [128,1024] pieces.
            nc.sync.dma_start(out=qT_sb[:], in_=qT[:])
            nc.sync.dma_start(out=onesd_sb[:], in_=onesd[:].bitcast(F32R))
            nc.sync.dma_start(out=kT_sb[0][:, 0:256], in_=kT[:, 0:256])
            nc.sync.dma_start(out=vt_sb[0][:, 0:256], in_=vt[:, 0:256])
            nc.sync.dma_start(out=kT_sb[0][:, 256:HW], in_=kT[:, 256:HW])
            nc.sync.dma_start(out=vt_sb[0][:, 256:HW], in_=vt[:, 256:HW])
            for i in range(1, 8):
                nc.sync.dma_start(out=kT_sb[i][:], in_=kT[:, i * HW : (i + 1) * HW])
                nc.sync.dma_start(out=vt_sb[i][:], in_=vt[:, i * HW : (i + 1) * HW])

            o_psum = apsum.tile([D, HW], F32, tag="o_psum")

            rs_acc = persist.tile([D, HW], F32, tag="rs_acc")
            nc.vector.memset(rs_acc[:], 0.0)

            def emit_qk(c):
                blk, off = c // 8, (c % 8) * 128
                s_ps = spsum.tile([D, HW], F32, tag="s_ps", name=f"s_ps{c}")
                for h in range(2):
                    nc.tensor.matmul(
                        s_ps[:, h * 512 : (h + 1) * 512],
                        kT_sb[blk][:, off : off + 128],
                        qT_sb[:, h * 512 : (h + 1) * 512],
                        start=True,
                        stop=True,
                    )
                return s_ps

            # Software-pipelined by one chunk: PE's program order is
            # QK(c+1) -> PV(c), so PE streams QK(c+1) while ACT exps S(c).
            s_tiles = {0: emit_qk(0)}
            e_tiles = {}
            for c in range(NCHUNK):
                if c + 1 < NCHUNK:
                    s_tiles[c + 1] = emit_qk(c + 1)

                e_sb = epool.tile([D, HW], BF16, tag="e_sb", name=f"e_sb{c}")
                nc.scalar.activation(
                    e_sb[:],
                    s_tiles.pop(c)[:],
                    mybir.ActivationFunctionType.Exp,
                    scale=SCALE,
                )
                e_tiles[c] = e_sb

                blk, off = c // 8, (c % 8) * 128
                for h in range(2):
                    nc.tensor.matmul(
                        o_psum[:, h * 512 : (h + 1) * 512],
                        vt_sb[blk][:, off : off + 128],
                        e_sb[:, h * 512 : (h + 1) * 512],
                        start=(c == 0),
                        stop=(c == NCHUNK - 1),
                    )

                # Rowsum on DVE only: bf16 pair-add (2x_1p mode) then fp32
                # accumulate, ~800 ns/chunk, under ACT's 1038 ns/chunk.
                if c % 2 == 1:
                    pair = ppool.tile([D, HW], BF16, tag="pair", name=f"pair{c}")
                    with nc.allow_low_precision(
                        reason="bf16 pair-sum of exp weights; accumulated in fp32"
                    ):
                        nc.vector.tensor_add(
                            pair[:], e_tiles.pop(c - 1)[:], e_tiles.pop(c)[:]
                        )
                        nc.vector.tensor_add(rs_acc[:], rs_acc[:], pair[:])

            # ---- epilogue: softmax denominator + normalize, 4 strips ----
            rs_acc_r = persist.tile([D, HW], F32R, tag="rs_acc_r")
            rs_fold = spsum.tile([D, HW], F32, tag="s_ps", name="rs_fold")
            rs_ps = rs_fold[0:1, :]
            rs_sb = persist.tile([1, HW], F32, tag="rs_sb")
            recip = persist.tile([1, HW], F32, tag="recip")
            bc_sb = persist.tile([D, HW], F32, tag="bc_sb")
            o_sb = persist.tile([D, HW], F32, tag="o_sb")
            for s4 in range(NSTRIP):
                sl = slice(SW * s4, SW * (s4 + 1))
                # f32r copy so the fp32r matmul verifier sees a rounded
                # producer, then fold partition-sums of rs_acc on PE
                nc.scalar.copy(rs_acc_r[:, sl], rs_acc[:, sl])
                nc.tensor.matmul(
                    rs_ps[:, sl],
                    onesd_sb[:],
                    rs_acc_r[:, sl],
                    start=True,
                    stop=True,
                )
                nc.scalar.copy(rs_sb[:, sl], rs_ps[:, sl])
                nc.vector.reciprocal(recip[:, sl], rs_sb[:, sl])
                nc.gpsimd.partition_broadcast(bc_sb[:, sl], recip[:, sl])
                nc.vector.tensor_mul(o_sb[:, sl], o_psum[:, sl], bc_sb[:, sl])
                nc.sync.dma_start(out=oT[:, sl], in_=o_sb[:, sl])

    nc.compile()
    return nc


_NC_CACHE = None


def _get_nc():
    global _NC_CACHE
    if _NC_CACHE is None:
        _NC_CACHE = _build_nc()
    return _NC_CACHE


def _prep_inputs(x: np.ndarray):
    x = np.ascontiguousarray(x, dtype=np.float32)
    xr = x.reshape(B, C, HW)

    # K channel-major over all tokens: kT[d, b*1024+hw] = x[b, 128+d, hw]
    kT = np.ascontiguousarray(xr[:, 128:256, :].transpose(1, 0, 2)).reshape(D, N)
    # V chunk-transposed: vt[p, 128*j + v] = V[128*j + p, v],
    # V[n, v] = x[b, v, hw] with n = b*1024 + hw
    v_tok = np.ascontiguousarray(xr[:, 0:128, :].transpose(0, 2, 1)).reshape(N, D)
    vt = np.ascontiguousarray(v_tok.reshape(NCHUNK, 128, D).transpose(1, 0, 2)).reshape(
        D, N
    )
    kT = kT.astype(ml_dtypes.bfloat16)
    vt = vt.astype(ml_dtypes.bfloat16)

    onesd = np.ones((D, 1), dtype=np.float32)
    in_maps = []
    for c in range(N_CORES):
        qT = np.ascontiguousarray(xr[c, 256:384, :]).astype(ml_dtypes.bfloat16)
        in_maps.append(
            {"qT": qT, "kT": kT, "vt": vt, "onesd": onesd}
        )
    return in_maps


def kernel(x: np.ndarray) -> np.ndarray:
    assert x.shape == (B, C, H, W), x.shape
    in_maps = _prep_inputs(x)
    nc = _get_nc()
    res = run_bass_kernel_spmd(nc, in_maps, list(range(N_CORES)))

    out = np.empty((B, D, H, W), dtype=np.float32)
    for c in range(N_CORES):
        out[c] = np.asarray(res.results[c]["oT"], dtype=np.float32).reshape(D, H, W)
    return out


# revision 8
# speedup vs baseline: 1.1726x; 1.0102x over previous
"""Global-attention kernel for [8, 384, 32, 32] ConvAttention on 8 trn2 cores.

Math (per reference): tokens over B*H*W = 8192 positions, C = 384 channels
split as V/K/Q of 128 each; out = softmax(Q K^T / sqrt(128)) V, re-laid as
[B, 128, H, W].

Sharding: core c owns the 1024 query tokens of batch c (token n = b*1024+hw,
so batch == contiguous token block). K/V are replicated. Each core computes
its row block of the attention entirely locally; no collectives.

v2 engine balance (per 128-kv-token chunk, 64 chunks):
  PE : QK (2x512 mm) + PV (2x512 mm)          ~853 ns   (bf16, 1 cyc/row)
  ACT: exp [128,1024] PSUM->SBUF bf16         ~1038 ns  <- critical path
  DVE: rowsum via bf16 pair-add (533) + fp32 accumulate (1067) per pair
       ~800 ns/chunk
  PE rowsum/ones-matmuls removed from the main loop entirely; epilogue is
  pipelined in 4 column strips (fold-mm -> ACT copy -> recip -> PE
  broadcast-mm -> DVE mul -> DMA out).

All matmul inputs are bf16 (host-converted), halving input DMA; first K/V
pieces are small so the first QK starts ~3us in.
"""

import math

import numpy as np
import ml_dtypes

import concourse.bass as bass
import concourse.tile as tile
from concourse import bacc, mybir
from concourse.bass_utils import run_bass_kernel_spmd

N_CORES = 8
B, C, H, W = 8, 384, 32, 32
HW = H * W            # 1024 tokens per batch == per core
N = B * HW            # 8192 total tokens
D = 128               # key/value width
NCHUNK = N // 128     # 64 kv chunks of 128 tokens
SCALE = 1.0 / math.sqrt(D)
F32 = mybir.dt.float32
F32R = mybir.dt.float32r
BF16 = mybir.dt.bfloat16

NSTRIP = 4            # epilogue column strips
SW = HW // NSTRIP


def _build_nc():
    nc = bacc.Bacc(
        "TRN2", target_bir_lowering=False, debug=False, num_devices=N_CORES
    )
    qT = nc.dram_tensor("qT", [D, HW], BF16, kind="ExternalInput").ap()
    kT = nc.dram_tensor("kT", [D, N], BF16, kind="ExternalInput").ap()
    vt = nc.dram_tensor("vt", [D, N], BF16, kind="ExternalInput").ap()
    onesd = nc.dram_tensor("onesd", [D, 1], F32, kind="ExternalInput").ap()
    oT = nc.dram_tensor("oT", [D, HW], F32, kind="ExternalOutput").ap()

    with tile.TileContext(nc) as tc:
        with (
            tc.tile_pool(name="persist", bufs=1) as persist,
            tc.tile_pool(name="etile", bufs=6) as epool,
            tc.tile_pool(name="pair", bufs=2) as ppool,
            tc.tile_pool(name="spsum", bufs=3, space="PSUM") as spsum,
            tc.tile_pool(name="apsum", bufs=1, space="PSUM") as apsum,
        ):
            qT_sb = persist.tile([D, HW], BF16, tag="qT_sb")
            onesd_sb = persist.tile([D, 1], F32, tag="onesd_sb")
            kT_sb = [persist.tile([D, HW], BF16, tag=f"kT{i}", name=f"kT_sb{i}") for i in range(8)]
            vt_sb = [persist.tile([D, HW], BF16, tag=f"vt{i}", name=f"vt_sb{i}") for i in range(8)]

            # Startup-latency-ordered DMA: Q first, then one small K piece
            # (2 chunks) and a small V piece so compute starts ~3us in, then
            # the bulk in [128,1024] pieces.
            nc.sync.dma_start(out=qT_sb[:, 0:512], in_=qT[:, 0:512])
            nc.sync.dma_start(out=kT_sb[0][:, 0:256], in_=kT[:, 0:256])
            nc.sync.dma_start(out=qT_sb[:, 512:HW], in_=qT[:, 512:HW])
            nc.sync.dma_start(out=vt_sb[0][:, 0:256], in_=vt[:, 0:256])
            nc.sync.dma_start(out=onesd_sb[:], in_=onesd[:])
            for j in range(1, 4):
                nc.sync.dma_start(
                    out=kT_sb[0][:, j * 256 : (j + 1) * 256],
                    in_=kT[:, j * 256 : (j + 1) * 256],
                )
                nc.sync.dma_start(
                    out=vt_sb[0][:, j * 256 : (j + 1) * 256],
                    in_=vt[:, j * 256 : (j + 1) * 256],
                )
            for i in range(1, 8):
                nc.sync.dma_start(out=kT_sb[i][:], in_=kT[:, i * HW : (i + 1) * HW])
                nc.sync.dma_start(out=vt_sb[i][:], in_=vt[:, i * HW : (i + 1) * HW])

            o_psum = apsum.tile([D, HW], F32, tag="o_psum")

            rs_acc = persist.tile([D, HW], F32, tag="rs_acc")
            nc.vector.memset(rs_acc[:], 0.0)

            def emit_qk(c):
                blk, off = c // 8, (c % 8) * 128
                s_ps = spsum.tile([D, HW], F32, tag="s_ps", name=f"s_ps{c}")
                for h in range(2):
                    nc.tensor.matmul(
                        s_ps[:, h * 512 : (h + 1) * 512],
                        kT_sb[blk][:, off : off + 128],
                        qT_sb[:, h * 512 : (h + 1) * 512],
                        start=True,
                        stop=True,
                    )
                return s_ps

            # Software-pipelined by one chunk: PE's program order is
            # QK(c+1) -> PV(c), so PE streams QK(c+1) while ACT exps S(c).
            s_tiles = {0: emit_qk(0)}
            e_tiles = {}
            for c in range(NCHUNK):
                if c + 1 < NCHUNK:
                    s_tiles[c + 1] = emit_qk(c + 1)

                e_sb = epool.tile([D, HW], BF16, tag="e_sb", name=f"e_sb{c}")
                nc.scalar.activation(
                    e_sb[:],
                    s_tiles.pop(c)[:],
                    mybir.ActivationFunctionType.Exp,
                    scale=SCALE,
                )
                e_tiles[c] = e_sb

                blk, off = c // 8, (c % 8) * 128
                for h in range(2):
                    nc.tensor.matmul(
                        o_psum[:, h * 512 : (h + 1) * 512],
                        vt_sb[blk][:, off : off + 128],
                        e_sb[:, h * 512 : (h + 1) * 512],
                        start=(c == 0),
                        stop=(c == NCHUNK - 1),
                    )

                # Rowsum on DVE only: bf16 pair-add (2x_1p mode) then fp32
                # accumulate, ~800 ns/chunk, under ACT's 1038 ns/chunk.
                if c >= NCHUNK - 2:
                    # last two chunks: direct accumulate so rs_acc is ready
                    # right after the last exp (shorter epilogue chain)
                    nc.vector.tensor_add(rs_acc[:], rs_acc[:], e_tiles.pop(c)[:])
                elif c % 2 == 1:
                    pair = ppool.tile([D, HW], BF16, tag="pair", name=f"pair{c}")
                    with nc.allow_low_precision(
                        reason="bf16 pair-sum of exp weights; accumulated in fp32"
                    ):
                        nc.vector.tensor_add(
                            pair[:], e_tiles.pop(c - 1)[:], e_tiles.pop(c)[:]
                        )
                        nc.vector.tensor_add(rs_acc[:], rs_acc[:], pair[:])

            # ---- epilogue: softmax denominator + normalize, 4 strips ----
            rs_fold = spsum.tile([D, HW], F32, tag="s_ps", name="rs_fold")
            rs_ps = rs_fold[0:1, :]
            recip = persist.tile([1, HW], F32, tag="recip")
            bc_sb = persist.tile([D, HW], F32, tag="bc_sb")
            o_sb = persist.tile([D, HW], F32, tag="o_sb")
            for s4 in range(NSTRIP):
                sl = slice(SW * s4, SW * (s4 + 1))
                # fp32 fold of the partition-sums on the (idle) PE; 4x
                # cycles/row but off the critical chain vs f32r copies
                nc.tensor.matmul(
                    rs_ps[:, sl],
                    onesd_sb[:],
                    rs_acc[:, sl],
                    start=True,
                    stop=True,
                )
                nc.vector.reciprocal(recip[:, sl], rs_ps[:, sl])
                nc.gpsimd.partition_broadcast(bc_sb[:, sl], recip[:, sl])
                nc.vector.tensor_mul(o_sb[:, sl], o_psum[:, sl], bc_sb[:, sl])
                nc.sync.dma_start(out=oT[:, sl], in_=o_sb[:, sl])

    nc.compile()
    return nc


_NC_CACHE = None


def _get_nc():
    global _NC_CACHE
    if _NC_CACHE is None:
        _NC_CACHE = _build_nc()
    return _NC_CACHE


def _prep_inputs(x: np.ndarray):
    x = np.ascontiguousarray(x, dtype=np.float32)
    xr = x.reshape(B, C, HW)

    # K channel-major over all tokens: kT[d, b*1024+hw] = x[b, 128+d, hw]
    kT = np.ascontiguousarray(xr[:, 128:256, :].transpose(1, 0, 2)).reshape(D, N)
    # V chunk-transposed: vt[p, 128*j + v] = V[128*j + p, v],
    # V[n, v] = x[b, v, hw] with n = b*1024 + hw
    v_tok = np.ascontiguousarray(xr[:, 0:128, :].transpose(0, 2, 1)).reshape(N, D)
    vt = np.ascontiguousarray(v_tok.reshape(NCHUNK, 128, D).transpose(1, 0, 2)).reshape(
        D, N
    )
    kT = kT.astype(ml_dtypes.bfloat16)
    vt = vt.astype(ml_dtypes.bfloat16)

    onesd = np.ones((D, 1), dtype=np.float32)
    in_maps = []
    for c in range(N_CORES):
        qT = np.ascontiguousarray(xr[c, 256:384, :]).astype(ml_dtypes.bfloat16)
        in_maps.append(
            {"qT": qT, "kT": kT, "vt": vt, "onesd": onesd}
        )
    return in_maps


def kernel(x: np.ndarray) -> np.ndarray:
    assert x.shape == (B, C, H, W), x.shape
    in_maps = _prep_inputs(x)
    nc = _get_nc()
    res = run_bass_kernel_spmd(nc, in_maps, list(range(N_CORES)))

    out = np.empty((B, D, H, W), dtype=np.float32)
    for c in range(N_CORES):
        out[c] = np.asarray(res.results[c]["oT"], dtype=np.float32).reshape(D, H, W)
    return out


# revision 10
# speedup vs baseline: 1.1850x; 1.0106x over previous
"""Global-attention kernel for [8, 384, 32, 32] ConvAttention on 8 trn2 cores.

Math (per reference): tokens over B*H*W = 8192 positions, C = 384 channels
split as V/K/Q of 128 each; out = softmax(Q K^T / sqrt(128)) V, re-laid as
[B, 128, H, W].

Sharding: core c owns the 1024 query tokens of batch c (token n = b*1024+hw,
so batch == contiguous token block). K/V are replicated. Each core computes
its row block of the attention entirely locally; no collectives.

v2 engine balance (per 128-kv-token chunk, 64 chunks):
  PE : QK (2x512 mm) + PV (2x512 mm)          ~853 ns   (bf16, 1 cyc/row)
  ACT: exp [128,1024] PSUM->SBUF bf16         ~1038 ns  <- critical path
  DVE: rowsum via bf16 pair-add (533) + fp32 accumulate (1067) per pair
       ~800 ns/chunk
  PE rowsum/ones-matmuls removed from the main loop entirely; epilogue is
  pipelined in 4 column strips (fold-mm -> ACT copy -> recip -> PE
  broadcast-mm -> DVE mul -> DMA out).

All matmul inputs are bf16 (host-converted), halving input DMA; first K/V
pieces are small so the first QK starts ~3us in.
"""

import math

import numpy as np
import ml_dtypes

import concourse.bass as bass
import concourse.tile as tile
from concourse import bacc, mybir
from concourse.bass_utils import run_bass_kernel_spmd

N_CORES = 8
B, C, H, W = 8, 384, 32, 32
HW = H * W            # 1024 tokens per batch == per core
N = B * HW            # 8192 total tokens
D = 128               # key/value width
NCHUNK = N // 128     # 64 kv chunks of 128 tokens
SCALE = 1.0 / math.sqrt(D)
F32 = mybir.dt.float32
F32R = mybir.dt.float32r
BF16 = mybir.dt.bfloat16

NSTRIP = 4            # epilogue column strips
SW = HW // NSTRIP


def _build_nc():
    nc = bacc.Bacc(
        "TRN2", target_bir_lowering=False, debug=False, num_devices=N_CORES
    )
    qT = nc.dram_tensor("qT", [D, HW], BF16, kind="ExternalInput").ap()
    kT = nc.dram_tensor("kT", [D, N], BF16, kind="ExternalInput").ap()
    vt = nc.dram_tensor("vt", [D, N], BF16, kind="ExternalInput").ap()
    onesd = nc.dram_tensor("onesd", [D, 1], F32, kind="ExternalInput").ap()
    oT = nc.dram_tensor("oT", [D, HW], F32, kind="ExternalOutput").ap()

    with tile.TileContext(nc) as tc:
        with (
            tc.tile_pool(name="persist", bufs=1) as persist,
            tc.tile_pool(name="etile", bufs=6) as epool,
            tc.tile_pool(name="pair", bufs=2) as ppool,
            tc.tile_pool(name="spsum", bufs=3, space="PSUM") as spsum,
            tc.tile_pool(name="apsum", bufs=1, space="PSUM") as apsum,
        ):
            qT_sb = persist.tile([D, HW], BF16, tag="qT_sb")
            onesd_sb = persist.tile([D, 1], F32, tag="onesd_sb")
            kT_sb = [persist.tile([D, HW], BF16, tag=f"kT{i}", name=f"kT_sb{i}") for i in range(8)]
            vt_sb = [persist.tile([D, HW], BF16, tag=f"vt{i}", name=f"vt_sb{i}") for i in range(8)]

            # Startup-latency-ordered DMA: Q first, then one small K piece
            # (2 chunks) and a small V piece so compute starts ~3us in, then
            # the bulk in [128,1024] pieces.
            nc.sync.dma_start(out=qT_sb[:, 0:512], in_=qT[:, 0:512])
            nc.sync.dma_start(out=kT_sb[0][:, 0:256], in_=kT[:, 0:256])
            nc.sync.dma_start(out=qT_sb[:, 512:HW], in_=qT[:, 512:HW])
            nc.sync.dma_start(out=vt_sb[0][:, 0:256], in_=vt[:, 0:256])
            nc.sync.dma_start(out=onesd_sb[:], in_=onesd[:])
            for j in range(1, 4):
                nc.sync.dma_start(
                    out=kT_sb[0][:, j * 256 : (j + 1) * 256],
                    in_=kT[:, j * 256 : (j + 1) * 256],
                )
                nc.sync.dma_start(
                    out=vt_sb[0][:, j * 256 : (j + 1) * 256],
                    in_=vt[:, j * 256 : (j + 1) * 256],
                )
            for i in range(1, 3):
                for j in range(2):
                    nc.sync.dma_start(
                        out=kT_sb[i][:, j * 512 : (j + 1) * 512],
                        in_=kT[:, i * HW + j * 512 : i * HW + (j + 1) * 512],
                    )
                    nc.sync.dma_start(
                        out=vt_sb[i][:, j * 512 : (j + 1) * 512],
                        in_=vt[:, i * HW + j * 512 : i * HW + (j + 1) * 512],
                    )
            for i in range(3, 8):
                nc.sync.dma_start(out=kT_sb[i][:], in_=kT[:, i * HW : (i + 1) * HW])
                nc.sync.dma_start(out=vt_sb[i][:], in_=vt[:, i * HW : (i + 1) * HW])

            o_psum = apsum.tile([D, HW], F32, tag="o_psum")

            rs_acc = persist.tile([D, HW], F32, tag="rs_acc")
            nc.vector.memset(rs_acc[:], 0.0)

            def emit_qk(c):
                blk, off = c // 8, (c % 8) * 128
                s_ps = spsum.tile([D, HW], F32, tag="s_ps", name=f"s_ps{c}")
                for h in range(2):
                    nc.tensor.matmul(
                        s_ps[:, h * 512 : (h + 1) * 512],
                        kT_sb[blk][:, off : off + 128],
                        qT_sb[:, h * 512 : (h + 1) * 512],
                        start=True,
                        stop=True,
                    )
                return s_ps

            # Software-pipelined by one chunk: PE's program order is
            # QK(c+1) -> PV(c), so PE streams QK(c+1) while ACT exps S(c).
            s_tiles = {0: emit_qk(0)}
            e_tiles = {}
            for c in range(NCHUNK):
                if c + 1 < NCHUNK:
                    s_tiles[c + 1] = emit_qk(c + 1)

                e_sb = epool.tile([D, HW], BF16, tag="e_sb", name=f"e_sb{c}")
                nc.scalar.activation(
                    e_sb[:],
                    s_tiles.pop(c)[:],
                    mybir.ActivationFunctionType.Exp,
                    scale=SCALE,
                )
                e_tiles[c] = e_sb

                blk, off = c // 8, (c % 8) * 128
                for h in range(2):
                    nc.tensor.matmul(
                        o_psum[:, h * 512 : (h + 1) * 512],
                        vt_sb[blk][:, off : off + 128],
                        e_sb[:, h * 512 : (h + 1) * 512],
                        start=(c == 0),
                        stop=(c == NCHUNK - 1),
                    )

                # Rowsum on DVE only: bf16 pair-add (2x_1p mode) then fp32
                # accumulate, ~800 ns/chunk, under ACT's 1038 ns/chunk.
                if c >= NCHUNK - 2:
                    # last two chunks: per-strip accumulates so each strip of
                    # rs_acc completes right after the last exp
                    e_last = e_tiles.pop(c)
                    for s4 in range(NSTRIP):
                        sl = slice(SW * s4, SW * (s4 + 1))
                        nc.vector.tensor_add(
                            rs_acc[:, sl], rs_acc[:, sl], e_last[:, sl]
                        )
                elif c % 2 == 1:
                    pair = ppool.tile([D, HW], BF16, tag="pair", name=f"pair{c}")
                    with nc.allow_low_precision(
                        reason="bf16 pair-sum of exp weights; accumulated in fp32"
                    ):
                        nc.vector.tensor_add(
                            pair[:], e_tiles.pop(c - 1)[:], e_tiles.pop(c)[:]
                        )
                        nc.vector.tensor_add(rs_acc[:], rs_acc[:], pair[:])

            # ---- epilogue: softmax denominator + normalize, 4 strips ----
            rs_fold = spsum.tile([D, HW], F32, tag="s_ps", name="rs_fold")
            rs_ps = rs_fold[0:1, :]
            recip = persist.tile([1, HW], F32, tag="recip")
            bc_sb = persist.tile([D, HW], F32, tag="bc_sb")
            o_sb = persist.tile([D, HW], F32, tag="o_sb")
            sls = [slice(SW * s4, SW * (s4 + 1)) for s4 in range(NSTRIP)]
            # stage-major emission keeps each engine's FIFO unblocked
            for sl in sls:
                # fp32 fold of the partition-sums on the (idle) PE; 4x
                # cycles/row but off the critical chain vs f32r copies
                nc.tensor.matmul(
                    rs_ps[:, sl], onesd_sb[:], rs_acc[:, sl], start=True, stop=True
                )
            for sl in sls:
                nc.vector.reciprocal(recip[:, sl], rs_ps[:, sl])
            for sl in sls:
                nc.gpsimd.partition_broadcast(bc_sb[:, sl], recip[:, sl])
            dma_qs = [nc.sync, nc.scalar, nc.gpsimd, nc.sync]
            for s4, sl in enumerate(sls):
                nc.vector.tensor_mul(o_sb[:, sl], o_psum[:, sl], bc_sb[:, sl])
                dma_qs[s4 % 4].dma_start(out=oT[:, sl], in_=o_sb[:, sl])

    nc.compile()
    return nc


_NC_CACHE = None


def _get_nc():
    global _NC_CACHE
    if _NC_CACHE is None:
        _NC_CACHE = _build_nc()
    return _NC_CACHE


def _prep_inputs(x: np.ndarray):
    x = np.ascontiguousarray(x, dtype=np.float32)
    xr = x.reshape(B, C, HW)

    # K channel-major over all tokens: kT[d, b*1024+hw] = x[b, 128+d, hw]
    kT = np.ascontiguousarray(xr[:, 128:256, :].transpose(1, 0, 2)).reshape(D, N)
    # V chunk-transposed: vt[p, 128*j + v] = V[128*j + p, v],
    # V[n, v] = x[b, v, hw] with n = b*1024 + hw
    v_tok = np.ascontiguousarray(xr[:, 0:128, :].transpose(0, 2, 1)).reshape(N, D)
    vt = np.ascontiguousarray(v_tok.reshape(NCHUNK, 128, D).transpose(1, 0, 2)).reshape(
        D, N
    )
    kT = kT.astype(ml_dtypes.bfloat16)
    vt = vt.astype(ml_dtypes.bfloat16)

    onesd = np.ones((D, 1), dtype=np.float32)
    in_maps = []
    for c in range(N_CORES):
        qT = np.ascontiguousarray(xr[c, 256:384, :]).astype(ml_dtypes.bfloat16)
        in_maps.append(
            {"qT": qT, "kT": kT, "vt": vt, "onesd": onesd}
        )
    return in_maps


def kernel(x: np.ndarray) -> np.ndarray:
    assert x.shape == (B, C, H, W), x.shape
    in_maps = _prep_inputs(x)
    nc = _get_nc()
    res = run_bass_kernel_spmd(nc, in_maps, list(range(N_CORES)))

    out = np.empty((B, D, H, W), dtype=np.float32)
    for c in range(N_CORES):
        out[c] = np.asarray(res.results[c]["oT"], dtype=np.float32).reshape(D, H, W)
    return out
